# revision 15
# baseline (speedup 1.0000x reference)
"""MoE block (small MLP on all-token-complement, big widened MLP on masked tokens)
as an 8-core Trainium2 Bass/Tile kernel.

Strategy: host-side routing + data parallelism.  The reference computes BOTH
experts densely on every token and selects with the mask; mathematically only
the selected expert's output is needed per token.  We gather big-expert tokens
(mask=True) and small-expert tokens (mask=False) on the host, deal them evenly
across the 8 NeuronCores, run both experts' MLPs on their respective token
shards (dense matmuls in bf16, fp32 accumulation), and scatter back.

Per-core layouts keep the contraction dim on SBUF partitions:
  x   : [d, t]   (d-chunks of 128 on partitions, tokens on the free dim)
  h   : [f, t]   (fc output produced directly in proj's required layout)
  out : [d, t]   (transposed back on the host)
so no on-chip transposes are needed anywhere.
"""

import math
import os

import numpy as np
import ml_dtypes

import concourse.bass as bass
import concourse.mybir as mybir
import concourse.tile as tile
from concourse import bacc
from concourse.bass_utils import run_bass_kernel_spmd

BF16 = ml_dtypes.bfloat16
N_CORES = 8
D_MODEL = 1024
D_FF_S = 4096
D_FF_B = 16384
KD = D_MODEL // 128        # 8 contraction chunks for fc
G = 8                      # f-chunks (of 128) per weight group
MAX_BLK = 512              # PSUM bank limit (fp32 free dim)

_nc_cache = {}
_weights_cache = {}


def _cap(n):
    """tokens-per-core capacity -> (cap, nblk, blk)."""
    t = max(1, math.ceil(n / N_CORES))
    nblk = max(1, math.ceil(t / MAX_BLK))
    blk = math.ceil(t / nblk)
    return nblk * blk, nblk, blk


def _build_nc(tb, nblk_b, blk_b, ts, nblk_s, blk_s):
    fcb = D_FF_B // 128
    fcs = D_FF_S // 128
    ngb = fcb // G
    ngs = fcs // G

    nc = bacc.Bacc("TRN2", target_bir_lowering=False, debug=False,
                   num_devices=N_CORES)
    dt = mybir.dt

    xb = nc.dram_tensor("xb", [KD, 128, tb], dt.bfloat16, kind="ExternalInput").ap()
    xs = nc.dram_tensor("xs", [KD, 128, ts], dt.bfloat16, kind="ExternalInput").ap()
    wfcb = nc.dram_tensor("wfcb", [ngb, 128, G, KD, 128], dt.bfloat16, kind="ExternalInput").ap()
    wpjb = nc.dram_tensor("wpjb", [ngb, 128, 8, G, 128], dt.bfloat16, kind="ExternalInput").ap()
    wfcs = nc.dram_tensor("wfcs", [ngs, 128, G, KD, 128], dt.bfloat16, kind="ExternalInput").ap()
    wpjs = nc.dram_tensor("wpjs", [ngs, 128, 8, G, 128], dt.bfloat16, kind="ExternalInput").ap()
    bfcb = nc.dram_tensor("bfcb", [128, fcb], dt.float32, kind="ExternalInput").ap()
    bfcs = nc.dram_tensor("bfcs", [128, fcs], dt.float32, kind="ExternalInput").ap()
    bpj = nc.dram_tensor("bpj", [128, 2, 8], dt.float32, kind="ExternalInput").ap()
    ob = nc.dram_tensor("ob", [D_MODEL, tb], dt.float32, kind="ExternalOutput").ap()
    os_ = nc.dram_tensor("os", [D_MODEL, ts], dt.float32, kind="ExternalOutput").ap()

    gelu = mybir.ActivationFunctionType.Gelu
    ident = mybir.ActivationFunctionType.Identity

    with tile.TileContext(nc) as tc:
        with (
            tc.tile_pool(name="xpool", bufs=2 * KD) as xpool,
            tc.tile_pool(name="wfc0", bufs=G) as wfc0_pool,
            tc.tile_pool(name="wfc", bufs=2) as wfc_pool,
            tc.tile_pool(name="wpj", bufs=2) as wpj_pool,
            tc.tile_pool(name="hpool", bufs=4 * G) as h_pool,
            tc.tile_pool(name="opool", bufs=16) as out_pool,
            tc.tile_pool(name="bias", bufs=1) as bias_pool,
            tc.tile_pool(name="ph", bufs=4, space="PSUM") as psum_h,
            tc.tile_pool(name="po", bufs=4, space="PSUM") as psum_o,
        ):
            # DMA-issue engine split (issue costs ~0.6us each and serialize
            # per engine): fc weights on the Sync HWDGE ring, proj weights /
            # x / biases / outputs on the Scalar HWDGE ring.

            # Startup critical path: the very first weight group is loaded
            # as G separate per-chunk tiles so the first matmuls start
            # after a 256KB transfer instead of the full 2MB group.
            wfc0_tiles = []
            for fl in range(G):
                w0 = wfc0_pool.tile([128, KD, 128], dt.bfloat16, tag="wfc0",
                                    name=f"wfc0_{fl}")
                nc.sync.dma_start(w0[:], wfcb[0, :, fl])
                wfc0_tiles.append(w0)

            def load_x(x_ap, tcap, prefix, engs):
                x_sb = []
                for k in range(KD):
                    xt = xpool.tile([128, tcap], dt.bfloat16, tag="x",
                                    name=f"x_{prefix}{k}")
                    engs[k % len(engs)].dma_start(xt[:], x_ap[k])
                    x_sb.append(xt)
                return x_sb

            xb_sb = load_x(xb, tb, "b", [nc.gpsimd, nc.scalar])

            bias_tiles = {}

            def emit_biases_impl():
                bias_tiles["bfcb"] = bias_pool.tile([128, fcb], dt.float32,
                                                    tag="bfcb", name="bfcb_sb")
                nc.scalar.dma_start(bias_tiles["bfcb"][:], bfcb)
                bias_tiles["bfcs"] = bias_pool.tile([128, fcs], dt.float32,
                                                    tag="bfcs", name="bfcs_sb")
                nc.scalar.dma_start(bias_tiles["bfcs"][:], bfcs)
                bias_tiles["bpj"] = bias_pool.tile([128, 2, 8], dt.float32,
                                                   tag="bpj", name="bpj_sb")
                nc.scalar.dma_start(bias_tiles["bpj"][:], bpj)

            emit_biases_impl()

            def expert(x_sb, w_fc_ap, w_pj_ap, bfc_key, bpj_col, out_ap,
                       tcap, nblk, blk, ng, first):
                out_sb = [out_pool.tile([128, tcap], dt.float32, tag="out",
                                        name=f"out_{bpj_col}{d}")
                          for d in range(8)]
                for fg in range(ng):
                    if first and fg == 0:
                        wfc_sl = lambda fl, k: wfc0_tiles[fl][:, k, :]
                    else:
                        wfc_t = wfc_pool.tile([128, G, KD, 128], dt.bfloat16,
                                              tag="wfc")
                        nc.sync.dma_start(wfc_t[:], w_fc_ap[fg])
                        wfc_sl = lambda fl, k, t=wfc_t: t[:, fl, k, :]
                    wpj_t = wpj_pool.tile([128, 8, G, 128], dt.bfloat16,
                                          tag="wpj")
                    nc.sync.dma_start(wpj_t[:], w_pj_ap[fg])
                    wpj_sl = lambda dd, fl, t=wpj_t: t[:, dd, fl, :]
                    bfc_sb = bias_tiles[bfc_key]
                    bpj_sb = bias_tiles["bpj"]
                    h = {}
                    for b in range(nblk):
                        tsl = bass.ts(b, blk)
                        for fl in range(G):
                            ph = psum_h.tile([128, blk], dt.float32, tag="ph")
                            for k in range(KD):
                                nc.tensor.matmul(ph[:], wfc_sl(fl, k),
                                                 x_sb[k][:, tsl],
                                                 start=(k == 0), stop=(k == KD - 1))
                            ht = h_pool.tile([128, blk], dt.bfloat16, tag="h")
                            fc = fg * G + fl
                            nc.scalar.activation(ht[:], ph[:], gelu,
                                                 bias=bfc_sb[:, fc:fc + 1])
                            h[b, fl] = ht
                    for d in range(8):
                        for b in range(nblk):
                            tsl = bass.ts(b, blk)
                            po = psum_o.tile([128, blk], dt.float32, tag="po")
                            for fl in range(G):
                                nc.tensor.matmul(po[:], wpj_sl(d, fl),
                                                 h[b, fl][:],
                                                 start=(fl == 0), stop=(fl == G - 1))
                            if fg == 0:
                                nc.scalar.activation(
                                    out_sb[d][:, tsl], po[:], ident,
                                    bias=bpj_sb[:, bpj_col, d:d + 1])
                            else:
                                nc.vector.tensor_add(out_sb[d][:, tsl],
                                                     out_sb[d][:, tsl], po[:])
                for d in range(8):
                    nc.scalar.dma_start(out_ap[d * 128:(d + 1) * 128, :],
                                        out_sb[d][:])

            expert(xb_sb, wfcb, wpjb, "bfcb", 0, ob, tb, nblk_b, blk_b, ngb, True)
            # second expert's x rides the Sync ring (idle after the fc
            # weight issues) so it never queues behind the gelu stream
            xs_sb = load_x(xs, ts, "s", [nc.gpsimd])
            expert(xs_sb, wfcs, wpjs, "bfcs", 1, os_, ts, nblk_s, blk_s, ngs, False)

    nc.compile()
    return nc


def _prep_weights(w_fc_s, b_fc_s, w_proj_s, b_proj_s, w_fc_b, b_fc_b,
                  w_proj_b, b_proj_b):
    key = (id(w_fc_s), id(w_fc_b), id(w_proj_s), id(w_proj_b))
    hit = _weights_cache.get(key)
    if hit is not None:
        return hit

    def fc_re(w, f):
        ng = f // 128 // G
        w16 = np.asarray(w, np.float32).astype(BF16)
        r = w16.reshape(ng, G, 128, KD, 128).transpose(0, 4, 1, 3, 2)
        return np.ascontiguousarray(r)

    def pj_re(w, f):
        ng = f // 128 // G
        w16 = np.asarray(w, np.float32).astype(BF16)
        r = w16.reshape(8, 128, ng, G, 128).transpose(2, 4, 0, 3, 1)
        return np.ascontiguousarray(r)

    def b_re(b, f):
        return np.ascontiguousarray(
            np.asarray(b, np.float32).reshape(f // 128, 128).T)

    bpj = np.stack([b_re(b_proj_b, D_MODEL), b_re(b_proj_s, D_MODEL)], axis=1)
    out = {
        "wfcb": fc_re(w_fc_b, D_FF_B),
        "wpjb": pj_re(w_proj_b, D_FF_B),
        "wfcs": fc_re(w_fc_s, D_FF_S),
        "wpjs": pj_re(w_proj_s, D_FF_S),
        "bfcb": b_re(b_fc_b, D_FF_B),
        "bfcs": b_re(b_fc_s, D_FF_S),
        "bpj": np.ascontiguousarray(bpj),
    }
    _weights_cache.clear()
    _weights_cache[key] = out
    return out


def kernel(x, mask, w_fc_s, b_fc_s, w_proj_s, b_proj_s,
           w_fc_b, b_fc_b, w_proj_b, b_proj_b, _profile=None):
    x = np.asarray(x, np.float32)
    mask = np.asarray(mask, bool)
    n_tok = x.shape[0] * x.shape[1]
    xf = x.reshape(n_tok, D_MODEL)
    mf = mask.reshape(n_tok)

    big_idx = np.nonzero(mf)[0]
    small_idx = np.nonzero(~mf)[0]
    tb, nblk_b, blk_b = _cap(len(big_idx))
    ts, nblk_s, blk_s = _cap(len(small_idx))

    def assign(idx, cap):
        a = np.full(N_CORES * cap, -1, np.int64)
        a[:len(idx)] = idx
        return a.reshape(N_CORES, cap)

    a_b = assign(big_idx, tb)
    a_s = assign(small_idx, ts)

    xf16 = xf.astype(BF16)

    def tok_arrays(a, cap):
        t = xf16[np.maximum(a, 0)]                       # [cores, cap, D]
        t = t.reshape(N_CORES, cap, KD, 128).transpose(0, 2, 3, 1)
        return np.ascontiguousarray(t)

    xb_all = tok_arrays(a_b, tb)
    xs_all = tok_arrays(a_s, ts)

    wd = _prep_weights(w_fc_s, b_fc_s, w_proj_s, b_proj_s,
                       w_fc_b, b_fc_b, w_proj_b, b_proj_b)

    nckey = (tb, nblk_b, blk_b, ts, nblk_s, blk_s)
    nc = _nc_cache.get(nckey)
    if nc is None:
        _nc_cache.clear()
        nc = _build_nc(*nckey)
        _nc_cache[nckey] = nc

    in_maps = [dict(wd, xb=xb_all[c], xs=xs_all[c]) for c in range(N_CORES)]
    kw = dict(_profile) if _profile else {}
    res = run_bass_kernel_spmd(nc, in_maps, core_ids=list(range(N_CORES)), **kw)

    out_t = np.empty((D_MODEL, n_tok), np.float32)

    def scatter(name, a):
        o = np.concatenate([res.results[c][name] for c in range(N_CORES)], axis=1)
        flat = a.reshape(-1)
        valid = flat >= 0
        out_t[:, flat[valid]] = o[:, valid]

    scatter("ob", a_b)
    scatter("os", a_s)

    out = out_t.T.reshape(x.shape)
    if _profile is not None:
        _profile["results"] = res
    return out
